# revision 31
# baseline (speedup 1.0000x reference)
"""Multi-head attention (B=4, S=2048, D=2048, H=16) on 8 trn2 NeuronCores.

Sharding: tensor-parallel over heads — 2 heads per core. Each core computes
its heads' Q/K/V projections, full attention for those heads, and a partial
output projection (its 256 rows of wo). The host sums the 8 partial outputs.

Schedule: one serpentine instruction stream keeps the PE dense end-to-end.
After batch-0's projections, each (batch, query-span) attention group carries
the NEXT batch's projection matmuls and the PREVIOUS span's out-projection
matmuls as explicit filler between score/AV pairs, so the PE never waits on
the ACT engine's exp latency. The softmax denominator is chunk-summed on the
DVE (bf16) and finished with a single ones-matmul per (span, head) instead of
16 — that removes ~100us of PE work vs accumulating it on the PE.

Precision: all matmul operands are bf16 (full PE rate, half the DMA/SBUF
bytes of fp32, fast FWL weight loads, 2x DVE modes); every accumulation —
projections, scores, AV, softmax denominator, out-proj — stays fp32 in PSUM.
Partial outputs ship bf16 and are summed in fp32 on the host.
Measured vs the fp32 reference: rel err ~7e-3 (gate 2e-2).
"""
import os
import sys

sys.path.insert(0, "/opt/trn_rl_repo")
import numpy as np

B, S, D, H = 4, 2048, 2048, 16
HD = 128
NCORES = 8
HP = H // NCORES          # heads per core = 2
DC = HP * HD              # per-core slice of D = 256
TOK = B * S               # 8192
SCALE = HD ** -0.5
NDC = D // 128            # 16 contraction chunks for the projections
SPAN = 512                # token span per projection step
NSPAN = S // SPAN         # 4 spans per batch
CSPL = 4                  # contraction chunks per x tile
NCT = NDC // CSPL         # 4 c-tiles per span
QS = 512                  # query span in attention
NQS = S // QS             # 4
NKC = S // 128            # 16 key chunks

LAST_EXEC_NS = None
_BUILT = None


def _round_tf32(x: np.ndarray) -> np.ndarray:
    """Round fp32 to tf32 (10 mantissa bits, RNE), keep fp32 container."""
    u = np.ascontiguousarray(x, dtype=np.float32).view(np.uint32)
    bias = np.uint32(0x00000FFF) + ((u >> np.uint32(13)) & np.uint32(1))
    return ((u + bias) & np.uint32(0xFFFFE000)).view(np.float32)


def _build():
    global _BUILT
    if _BUILT is not None:
        return _BUILT
    import concourse.tile as tile
    from concourse import bacc, mybir

    F32R = mybir.dt.float32r
    F32 = mybir.dt.float32
    BF16 = mybir.dt.bfloat16
    Exp = mybir.ActivationFunctionType.Exp
    Ident = mybir.ActivationFunctionType.Identity

    nc = bacc.Bacc("TRN2", target_bir_lowering=False, debug=False)
    xt = nc.dram_tensor("xt", [D, TOK], BF16, kind="ExternalInput")
    wq = nc.dram_tensor("wq", [D, DC], BF16, kind="ExternalInput")
    wk = nc.dram_tensor("wk", [D, DC], BF16, kind="ExternalInput")
    wv = nc.dram_tensor("wv", [D, DC], BF16, kind="ExternalInput")
    wo = nc.dram_tensor("wo", [DC, D], BF16, kind="ExternalInput")
    bq2 = nc.dram_tensor("bq2", [HD, HP], F32, kind="ExternalInput")
    bk2 = nc.dram_tensor("bk2", [HD, HP], F32, kind="ExternalInput")
    ones = nc.dram_tensor("ones", [128, 128], BF16, kind="ExternalInput")
    out = nc.dram_tensor("out", [TOK, D], BF16, kind="ExternalOutput")

    with tile.TileContext(nc) as tc:
        with tc.tile_pool(name="const", bufs=1) as cpool, \
             tc.tile_pool(name="xp", bufs=5) as xpool, \
             tc.tile_pool(name="bt", bufs=2) as bpool, \
             tc.tile_pool(name="at", bufs=2) as apool, \
             tc.tile_pool(name="ot", bufs=2) as opool, \
             tc.tile_pool(name="ps", bufs=1, space="PSUM") as ps:

            wq_sb = cpool.tile([128, NDC, DC], BF16)
            wk_sb = cpool.tile([128, NDC, DC], BF16)
            wv_sb = cpool.tile([128, NDC, DC], BF16)
            wo_sb = cpool.tile([128, HP, D], BF16)
            ones_sb = cpool.tile([128, 128], BF16)
            bq_sb = cpool.tile([HD, HP], F32)
            bk_sb = cpool.tile([HD, HP], F32)
            warm = cpool.tile([HD, 1], F32)

            xt_r = xt.rearrange("(c p) t -> p c t", p=128)

            # ---- DMA prologue ----
            wq_r = wq.rearrange("(c p) n -> p c n", p=128)
            wk_r = wk.rearrange("(c p) n -> p c n", p=128)
            # halves so the first Q matmuls start on a partial weight load
            nc.sync.dma_start(out=wq_sb[:, 0:NDC // 2], in_=wq_r[:, 0:NDC // 2])
            nc.sync.dma_start(out=bq_sb, in_=bq2[:, :])
            nc.sync.dma_start(out=bk_sb, in_=bk2[:, :])
            # preload the exp table set (~2.7us) under the projection phase
            nc.scalar.activation(warm, bq_sb[:, 0:1], Exp)

            # x tiles: [128, CSPL, SPAN] slices of xt, keyed (b, sp, ci);
            # DMA emission is pumped so the pool (bufs=5) stays ahead.
            xtiles = {}
            xorder = [(b, sp, ci) for b in range(B) for sp in range(NSPAN)
                      for ci in range(NCT)]
            xnext = [0]

            def pump_x(n):
                for _ in range(n):
                    if xnext[0] >= len(xorder):
                        return
                    b, sp, ci = xorder[xnext[0]]
                    xnext[0] += 1
                    t0 = b * S + sp * SPAN
                    xti = xpool.tile([128, CSPL, SPAN], BF16, name="xti",
                                     tag="xsp", bufs=12)
                    nc.sync.dma_start(
                        out=xti,
                        in_=xt_r[:, ci * CSPL:(ci + 1) * CSPL, t0:t0 + SPAN])
                    xtiles[(b, sp, ci)] = xti

            pump_x(4)                 # span (0,0) fully
            nc.sync.dma_start(out=wq_sb[:, NDC // 2:], in_=wq_r[:, NDC // 2:])
            nc.sync.dma_start(out=wk_sb[:, 0:NDC // 2], in_=wk_r[:, 0:NDC // 2])
            nc.sync.dma_start(out=wk_sb[:, NDC // 2:], in_=wk_r[:, NDC // 2:])
            nc.sync.dma_start(out=wv_sb, in_=wv.rearrange("(c p) n -> p c n", p=128))
            pump_x(1)                 # first tile of span (0,1)
            nc.sync.dma_start(out=wo_sb, in_=wo.rearrange("(c p) n -> p c n", p=128))
            nc.sync.dma_start(out=ones_sb, in_=ones[:, :])

            # per-batch SBUF tensors (bufs=2 → double-buffered across batches)
            qt = {}
            kt = {}
            vb = {}

            def batch_tiles(b):
                if b not in qt:
                    qt[b] = bpool.tile([128, HP, S], BF16, name=f"qt{b}",
                                       tag="qt", bufs=2)
                    kt[b] = bpool.tile([128, HP, S], BF16, name=f"kt{b}",
                                       tag="kt", bufs=2)
                    vb[b] = bpool.tile([128, NKC, DC], BF16, name=f"v{b}",
                                       tag="v", bufs=2)
                return qt[b], kt[b], vb[b]

            # ---- projection unit: generator of filler chunks ----
            def proj_chunks(b, sp):
                qt_b, kt_b, v_b = batch_tiles(b)

                # Q/K sections: one PSUM bank per (proj, head), 16 matmuls N=512
                for wsb, dst, bias in ((wq_sb, qt_b, bq_sb), (wk_sb, kt_b, bk_sb)):
                    for h in range(HP):
                        pps = ps.tile([128, SPAN], F32, name="pps", tag="pj",
                                      bufs=2)
                        for ci in range(NCT):
                            def qk_chunk(ci=ci, pps=pps, wsb=wsb, h=h):
                                xti = xtiles[(b, sp, ci)]
                                for cc in range(CSPL):
                                    c = ci * CSPL + cc
                                    nc.tensor.matmul(
                                        pps, wsb[:, c, h * HD:(h + 1) * HD],
                                        xti[:, cc, :],
                                        start=(c == 0), stop=(c == NDC - 1))
                            yield qk_chunk

                        def qk_drain(pps=pps, dst=dst, bias=bias, h=h):
                            nc.scalar.activation(
                                dst[:, h, sp * SPAN:(sp + 1) * SPAN], pps,
                                Ident, bias=bias[:, h:h + 1])
                            pump_x(1)
                        yield qk_drain

                # V sections: token-major, two 128-token chunks share a bank
                for pair in range(SPAN // 256):
                    vps = ps.tile([128, 512], F32, name="vps", tag="pj", bufs=2)
                    for tl in (2 * pair, 2 * pair + 1):
                        for ci in range(NCT):
                            def v_chunk(ci=ci, tl=tl, vps=vps):
                                xti = xtiles[(b, sp, ci)]
                                for cc in range(CSPL):
                                    c = ci * CSPL + cc
                                    nc.tensor.matmul(
                                        vps[:, (tl % 2) * DC:(tl % 2 + 1) * DC],
                                        xti[:, cc, tl * 128:(tl + 1) * 128],
                                        wv_sb[:, c, :],
                                        start=(c == 0), stop=(c == NDC - 1))
                            yield v_chunk

                    def v_drain(pair=pair, vps=vps, v_b=v_b):
                        kc0 = sp * (SPAN // 128) + 2 * pair
                        nc.scalar.copy(v_b[:, kc0:kc0 + 2, :], vps)
                    yield v_drain

            # ---- out-projection unit: generator of filler chunks ----
            def out_chunks(b, qs, avt):
                for tl in range(QS // 128):
                    tch = qs * (QS // 128) + tl
                    out_sb = opool.tile([128, D], BF16, name="out_sb",
                                        tag="out_sb", bufs=6)
                    for dsp in range(D // 512):
                        def o_chunk(tl=tl, dsp=dsp, out_sb=out_sb, avt=avt,
                                    tch=tch, last=(dsp == D // 512 - 1)):
                            ops = ps.tile([128, 512], F32, name="ops",
                                          tag="pj", bufs=2)
                            for h in range(HP):
                                nc.tensor.matmul(
                                    ops, avt[:, h, tl * 128:(tl + 1) * 128],
                                    wo_sb[:, h, dsp * 512:(dsp + 1) * 512],
                                    start=(h == 0), stop=(h == HP - 1))
                            # split drains across DVE and ACT so neither
                            # engine's FIFO backs up behind the copies
                            if dsp % 2 == 0:
                                nc.vector.tensor_copy(
                                    out_sb[:, dsp * 512:(dsp + 1) * 512], ops)
                            else:
                                nc.scalar.copy(
                                    out_sb[:, dsp * 512:(dsp + 1) * 512], ops)
                            if last:
                                nc.gpsimd.dma_start(
                                    out=out[b * S + tch * 128:
                                            b * S + (tch + 1) * 128, :],
                                    in_=out_sb)
                        yield o_chunk

            # ---- attention unit ----
            def att_unit(b, qs, h, pending, fill, avt):
                qt_b, kt_b, v_b = batch_tiles(b)
                q_sl = qt_b[:, h, qs * QS:(qs + 1) * QS]
                av_ps = ps.tile([HD, QS], F32, name="av_ps", tag="av", bufs=2)
                csum = apool.tile([128, 2 * QS], BF16, name="csum", tag="csum",
                                  bufs=2)
                p_tiles = []

                def av_pair(kp):
                    p_prev = p_tiles[kp]
                    for j in range(2):
                        kc = 2 * kp + j
                        nc.tensor.matmul(
                            av_ps, v_b[:, kc, h * HD:(h + 1) * HD],
                            p_prev[:, j * QS:(j + 1) * QS],
                            start=(kc == 0), stop=(kc == NKC - 1))

                for kp in range(NKC // 2):
                    s_ps = ps.tile([128, 2 * QS], F32, name="s_ps", tag="s",
                                   bufs=2)
                    for j in range(2):
                        kc = 2 * kp + j
                        nc.tensor.matmul(
                            s_ps[:, j * QS:(j + 1) * QS],
                            kt_b[:, h, kc * 128:(kc + 1) * 128], q_sl,
                            start=True, stop=True)
                    p_sb = apool.tile([128, 2 * QS], BF16, name="p_sb",
                                      tag="p", bufs=4)
                    nc.scalar.activation(p_sb, s_ps, Exp, scale=SCALE)
                    p_tiles.append(p_sb)
                    # last-batch groups have no projection drains to pace the
                    # DVE, which saturates there — push the chunk-sums to the
                    # otherwise-idle GPSIMD in those groups
                    veng = nc.gpsimd if b == B - 1 else nc.vector
                    if kp == 1:
                        veng.tensor_add(csum, p_tiles[0], p_tiles[1])
                    elif kp >= 2:
                        veng.tensor_add(csum, csum, p_tiles[kp])
                    if kp == 3 and pending is not None:
                        # previous unit's deferred normalization: late enough
                        # that the DVE has drained the csum chain it needs
                        # (no PE stall), and always before the first out-proj
                        # filler chunk that reads the avt it writes
                        pending()
                    if kp >= 1:
                        av_pair(kp - 1)
                    fill(kp)
                av_pair(NKC // 2 - 1)

                csf = apool.tile([128, QS], BF16, name="csf", tag="csf", bufs=2)
                nc.vector.tensor_add(csf, csum[:, 0:QS], csum[:, QS:2 * QS])

                def tail(_csf=csf, _av=av_ps, _avt=avt, _h=h):
                    dn_ps = ps.tile([128, QS], F32, name="dn_ps", tag="pj",
                                    bufs=2)
                    nc.tensor.matmul(dn_ps, ones_sb, _csf, start=True,
                                     stop=True)
                    recip = apool.tile([128, QS], F32, name="recip",
                                       tag="recip", bufs=2)
                    nc.vector.reciprocal_approx_fast(recip, dn_ps)
                    nc.vector.tensor_mul(_avt[:, _h, :], _av, recip)
                return tail

            # ---- serpentine ----
            # batch-0 projections run dense (nothing to overlap with yet)
            for sp in range(NSPAN):
                for ch in proj_chunks(0, sp):
                    ch()

            pending = None
            outq = []                 # (b, qs, avt) units ready for out-proj
            for b in range(B):
                for qs in range(NQS):
                    avt = apool.tile([128, HP, QS], BF16, name="avt",
                                     tag="avt", bufs=4)
                    fq = []
                    if b < B - 1:
                        fq.extend(proj_chunks(b + 1, qs))
                    # out-proj units lag their group; the last batch has no
                    # projection filler, so bank two units for each of its
                    # groups by under-consuming at the end of batch B-2
                    if b == B - 2 and qs >= 2:
                        npop = 0
                    elif b == B - 1:
                        npop = 2
                    else:
                        npop = 1
                    for _ in range(npop):
                        if outq:
                            fq.extend(out_chunks(*outq.pop(0)))
                    fq.reverse()      # pop() from the front of the list
                    nfill = 2 * (NKC // 2)
                    # when only out-proj chunks are queued, hold them back a
                    # couple of pairs so the deferred muls they read are done
                    skip = 4 if b == B - 1 else 0
                    per_pair = max(1, -(-len(fq) // (nfill - skip)))
                    state = [0]

                    def fill(kp, fq=fq, per_pair=per_pair, skip=skip,
                             state=state):
                        state[0] += 1
                        if state[0] <= skip:
                            return
                        for _ in range(per_pair):
                            if fq:
                                fq.pop()()

                    for h in range(HP):
                        pending = att_unit(b, qs, h, pending, fill, avt)
                    while fq:
                        fq.pop()()
                    outq.append((b, qs, avt))

            pending()
            while outq:
                for ch in out_chunks(*outq.pop(0)):
                    ch()

    nc.compile()
    _BUILT = nc
    return nc


def _install_trace_hooks():
    import types
    try:
        import antenv.axon_hooks  # noqa: F401
        return True
    except ImportError:
        pass
    try:
        from trn_agent_boot.trn_boot import _ntff_profile_via_ctypes
        hook = _ntff_profile_via_ctypes('/opt/axon/libaxon_pjrt.so')
        if hook is None:
            return False
        m = types.ModuleType('antenv.axon_hooks')
        m.get_axon_ntff_profile_hook = lambda: hook
        sys.modules['antenv.axon_hooks'] = m
        from concourse import bass_utils
        bass_utils.upload_artifacts = lambda tmpdir: "local://" + tmpdir
        return True
    except Exception:
        return False


def kernel(x, wq, bq, wk, bk, wv, bv, wo, bo):
    global LAST_EXEC_NS
    from concourse.bass_utils import run_bass_kernel_spmd

    x = np.asarray(x, dtype=np.float32)
    wq = np.asarray(wq, dtype=np.float32)
    bq = np.asarray(bq, dtype=np.float32)
    wk = np.asarray(wk, dtype=np.float32)
    bk = np.asarray(bk, dtype=np.float32)
    wv = np.asarray(wv, dtype=np.float32)
    bv = np.asarray(bv, dtype=np.float32)
    wo = np.asarray(wo, dtype=np.float32)
    bo = np.asarray(bo, dtype=np.float32)

    import ml_dtypes

    BF = ml_dtypes.bfloat16
    xt = np.ascontiguousarray(x.reshape(TOK, D).T).astype(BF)
    ones = np.ones((128, 128), dtype=BF)
    in_maps = []
    for i in range(NCORES):
        sl = slice(i * DC, (i + 1) * DC)
        in_maps.append({
            "xt": xt,
            "wq": np.ascontiguousarray(wq[:, sl]).astype(BF),
            "wk": np.ascontiguousarray(wk[:, sl]).astype(BF),
            "wv": np.ascontiguousarray(wv[:, sl]).astype(BF),
            "wo": np.ascontiguousarray(wo[sl, :]).astype(BF),
            "bq2": np.ascontiguousarray(bq[sl].reshape(HP, HD).T),
            "bk2": np.ascontiguousarray(bk[sl].reshape(HP, HD).T),
            "ones": ones,
        })

    trace = bool(os.environ.get("KERNEL_TRACE"))
    if trace:
        trace = _install_trace_hooks()

    nc = _build()
    res = run_bass_kernel_spmd(nc, in_maps, list(range(NCORES)), trace=trace)
    LAST_EXEC_NS = res.exec_time_ns

    total = np.zeros((TOK, D), dtype=np.float32)
    for r in res.results:
        total += np.asarray(r["out"]).astype(np.float32)
    # V-bias folds into a constant row: softmax rows sum to 1, so
    # attention(V + 1*bv^T) = attention(V) + 1*bv^T, and (bv @ wo) adds to bo.
    total += bo + bv @ wo
    return total.reshape(B, S, D)
